# revision 34
# baseline (speedup 1.0000x reference)
"""DRT scorer kernel for Trainium2 (8 NeuronCores, Bass/Tile).

score[b, p] = sum_k alpha[b,k] * <qsub[b,k,:], dsub[p,k,:]>
with qsub/dsub per-slot-L2-normalized outputs of a shared 2-layer MLP
(E=384 -> H=512 -> K*SUB=384) and alpha a softmax over an attention MLP.

Strategy:
  - Fold alpha and query norms into the query side: qmod[b, s] =
    alpha[b, s//64] * qsub_norm[b, s].  Then score = Dnorm @ qmod.T.
  - Shard docs P across 8 cores (data parallel), pad 100000 -> 101888
    (12736/core; +64 query columns = 12800 = 25 tiles x 512).
  - MM1 (E->H) in bf16; MM2 (H->K*SUB) in fp8e4m3 with DoubleRow
    double-pumping (H=512 = 4 k-tiles = 2 DoubleRow passes, 2x rate).
    fp8 on one layer keeps L2 rel err ~1.3e-2 (< 2e-2 gate); fp8 on
    both layers measured 1.93e-2 - too close.
  - Per-slot doc norms via a block-diagonal ones matmul (fp8 operands)
    producing partition-replicated norm^2; 1/sqrt via ACT Rsqrt.
  - Elementwise split across ACT (2 relu, 1 sn0, 3 rsqrt), DVE (2
    relu, 2 sn0, 3 sn-mul), Pool (3 sq-square, out copy) so no engine
    exceeds the PE's ~4.5us/tile.
  - Software pipeline: tile t's norm+score phase is emitted one tile
    late so the PE never stalls on the norm chain.
"""

import sys

sys.path.insert(0, "/opt/trn_rl_repo")

import ml_dtypes
import numpy as np
import concourse.bacc as bacc
import concourse.mybir as mybir
from concourse.tile import TileContext
from concourse.bass_utils import run_bass_kernel_spmd

F32 = mybir.dt.float32
BF16 = mybir.dt.bfloat16
F8 = mybir.dt.float8e4
AF = mybir.ActivationFunctionType
ALU = mybir.AluOpType
DR = mybir.MatmulPerfMode.DoubleRowSwInterleave

E, H, KSUB = 384, 512, 384
NSLOT, SUB = 6, 64
AH = 64
B = 64
P_FULL = 100000
N_CORES = 8
TILE = 512
P_SHARD = 12800  # columns per core = B queries + D_CORE docs
NT = P_SHARD // TILE  # 25
D_CORE = P_SHARD - B  # 12736 doc slots per core (8*12736 = 101888 >= P_FULL)
EB, HB, SB = E // 128, H // 128, KSUB // 128  # 3, 4, 3
PAIRS = HB // 2  # 2 DoubleRow k-tile pairs for MM2
EPS = 1e-12

_CACHE = {}


def _act_rsqrt(nc, out, in_, bias_ap):
    """out = 1/sqrt(in + bias) on the ACT engine.

    bass refuses AF.Rsqrt on accuracy grounds (~0.4% worst case); the
    score tolerance here is much looser and this keeps the doc loop on a
    single activation-table set (the DVE reciprocal alternative costs
    ~3.2us per 512-col tile, and sqrt/ln/exp sit in different table sets
    whose reloads cost ~2.7us each).
    """
    sc = nc.scalar
    ins = [
        sc.lower_ap(in_),
        sc.lower_ap(bias_ap),
        mybir.ImmediateValue(dtype=F32, value=1.0),
        mybir.ImmediateValue(dtype=F32, value=0.0),
    ]
    return sc.add_instruction(
        mybir.InstActivation(
            name=nc.get_next_instruction_name(),
            func=AF.Rsqrt,
            ins=ins,
            outs=[sc.lower_ap(out)],
        )
    )


def _consts():
    # mask[p, j] = 1 iff p//64 == j//64  (block-diagonal 64x64 ones)
    idx = np.arange(128)
    mask = (idx[:, None] // SUB == idx[None, :] // SUB).astype(np.float32)
    # sel[k, sb*128 + j] = 1 iff k == 2*sb + j//64
    sel = np.zeros((NSLOT, KSUB), dtype=np.float32)
    for sb in range(SB):
        for j in range(128):
            sel[2 * sb + j // SUB, sb * 128 + j] = 1.0
    ones6 = np.ones((NSLOT, 128), dtype=np.float32)
    return mask, sel, ones6


def build(nt=NT):
    p_shard = nt * TILE
    nc = bacc.Bacc()

    # column layout per core: [0:B] = query embeddings, [B:] = doc shard.
    # Tile 0's MLP+norm pipeline thereby computes the normalized query
    # sub-vectors for free; only the alpha MLP runs separately.
    # Tile-major DRAM layout: docs[t*128+p, eb*TILE+c] = emb[eb*128+p,
    # global col], so each tile's DMA is 128 x 3KB contiguous runs
    # (1KB strided runs measured ~3x slower at startup).
    docs = nc.declare_dram_parameter(
        "docs", [nt * 128, EB * TILE], BF16, isOutput=False
    )
    # weights pre-packed on host into the exact SBUF image. w1 rides its
    # own DMA so the first doc-tile matmuls aren't queued behind the rest.
    RCOLS = EB * AH + NSLOT
    w1pack = nc.declare_dram_parameter("w1pack", [128, EB * H], BF16, isOutput=False)
    # w2 as fp8 DoubleRow weight image: col = ((sb*2+pair)*2+i)*128 + m,
    # value = W2[(2*pair+i)*128 + p, sb*128 + m]
    w2pack = nc.declare_dram_parameter(
        "w2pack", [128, SB * PAIRS * 2 * 128], F8, isOutput=False
    )
    wrest = nc.declare_dram_parameter("wrest", [128, RCOLS], BF16, isOutput=False)
    # biases packed: cols [0:HB] b1, [HB:HB+SB] b2, [HB+SB] ba1, [HB+SB+1] ba2
    bpack = nc.declare_dram_parameter("bpack", [128, HB + SB + 2], F32, isOutput=False)
    scores = nc.declare_dram_parameter("scores", [B, p_shard], F32, isOutput=True)

    mask_np, sel_np, ones6_np = _consts()
    bf = ml_dtypes.bfloat16
    cpack_np = np.zeros((128, KSUB + 128), dtype=np.float32)
    cpack_np[:NSLOT, 0:KSUB] = sel_np
    cpack_np[:NSLOT, KSUB:] = ones6_np
    cpack_d = nc.inline_tensor(cpack_np.astype(bf), name="cpack_d")
    mask8_d = nc.inline_tensor(
        mask_np.astype(ml_dtypes.float8_e4m3), name="mask8_d"
    )

    with TileContext(nc) as tc:
        with (
            tc.tile_pool(name="consts", bufs=1) as consts,
            tc.tile_pool(name="qpool", bufs=1) as qpool,
            tc.tile_pool(name="xtp", bufs=6) as xtp,
            tc.tile_pool(name="htp", bufs=4) as htp,
            tc.tile_pool(name="sn0p", bufs=9) as sn0p,
            tc.tile_pool(name="sqp", bufs=9) as sqp,
            tc.tile_pool(name="rip", bufs=6) as rip,
            tc.tile_pool(name="snp", bufs=12) as snp,
            tc.tile_pool(name="outp", bufs=4) as outp,
            tc.tile_pool(name="psh", bufs=3, space="PSUM") as psh,
            tc.tile_pool(name="pss", bufs=2, space="PSUM") as pss,
            tc.tile_pool(name="psn", bufs=2, space="PSUM") as psn,
            tc.tile_pool(name="psc", bufs=1, space="PSUM") as psc,
        ):
            # ---- constants / weights to SBUF ----
            # w1 rides the first DMA in one shot (contiguous 3KB runs);
            # the first doc tiles follow immediately on their own queues.
            # First wave = ONLY what gates the first matmuls (bpack, w1,
            # xt0), split into partition quarters: each split owns a DMA
            # queue and the 16 HW engines round-robin across ACTIVE
            # queues, so the critical bytes get most of the early
            # (~100GB/s) aggregate. Everything else is emitted after
            # tile 0's MM1 so its Sync-engine triggers (and thus queue
            # activation) trail the critical wave.
            bt = consts.tile([128, HB + SB + 2], F32)
            nc.sync.dma_start(out=bt, in_=bpack[:, :])
            b1t = bt[:, 0:HB]
            b2t = bt[:, HB : HB + SB]
            ba1t = bt[:AH, HB + SB : HB + SB + 1]
            ba2t = bt[:NSLOT, HB + SB + 1 : HB + SB + 2]

            w1t = consts.tile([128, EB * H], BF16)
            for q in range(4):
                nc.sync.dma_start(
                    out=w1t[q * 32 : (q + 1) * 32, :],
                    in_=w1pack[q * 32 : (q + 1) * 32, :],
                )
            w1 = w1t[:, :].rearrange("p (eb h) -> p eb h", eb=EB)

            docs_r = docs[:, :].rearrange(
                "(t p) (eb c) -> t p eb c", p=128, eb=EB
            )
            xt_pre = {}
            xt0_t = xtp.tile([128, EB, TILE], BF16, tag="xt")
            for q in range(4):
                nc.sync.dma_start(
                    out=xt0_t[q * 32 : (q + 1) * 32],
                    in_=docs_r[0, q * 32 : (q + 1) * 32],
                )
            xt_pre[0] = xt0_t

            epst = consts.tile([128, 1], F32)
            nc.vector.memset(epst, EPS)

            # Deferred-wave tiles (DMAs emitted later, between phases)
            # SwInterleave weight image: per (sb, pair) the two k-tiles'
            # columns are pair-interleaved and reversed (A127,B127,...,
            # A0,B0) so the 256-row DR weight load is a single pass.
            w2t = consts.tile([128, SB * PAIRS * 2 * 128], F8)
            w2 = w2t[:, :].rearrange(
                "p (sb pr k i) -> p sb pr k i", sb=SB, pr=PAIRS, i=2
            )
            wrt = consts.tile([128, RCOLS], BF16)
            wa1 = wrt[:, 0 : EB * AH].rearrange("p (eb a) -> p eb a", eb=EB)
            wa2 = wrt[:AH, EB * AH :]
            ct = consts.tile([128, KSUB + 128], BF16)
            sel = ct[:NSLOT, 0:KSUB]
            ones6 = ct[:NSLOT, KSUB:]
            mt = consts.tile([128, 128], F8)
            mask = mt[:, :]

            def stage_a_mm1(t):
                # tile t: load + MM1 (bf16) + relu -> ht (fp8)
                if t in xt_pre:
                    xt = xt_pre.pop(t)
                else:
                    xt = xtp.tile([128, EB, TILE], BF16, tag="xt", name="xt")
                    nc.sync.dma_start(out=xt, in_=docs_r[t])
                ht = htp.tile([128, HB, TILE], F8, tag="ht", name="ht")
                for hb in range(HB):
                    h_ps = psh.tile([128, TILE], F32, tag="psh", name="h_ps")
                    for eb in range(EB):
                        nc.tensor.matmul(
                            h_ps,
                            w1[:, eb, hb * 128 : (hb + 1) * 128],
                            xt[:, eb, :],
                            start=(eb == 0),
                            stop=(eb == EB - 1),
                        )
                    if hb < 2:
                        nc.scalar.activation(
                            out=ht[:, hb, :], in_=h_ps, func=AF.Relu,
                            bias=b1t[:, hb : hb + 1],
                        )
                    else:
                        nc.vector.tensor_scalar(
                            out=ht[:, hb, :], in0=h_ps,
                            scalar1=b1t[:, hb : hb + 1],
                            scalar2=0.0, op0=ALU.add, op1=ALU.max,
                        )
                return xt, ht

            def stage_a_mm2(t, ht):
                # tile t: MM2 (fp8 DoubleRow) + sn0 + sq.
                # DR pairs for sb0/sb1 are interleaved across their two
                # PSUM banks so each 256-row DR LDWEIGHTS (~190ns, no
                # FWL) hides under a full matmul window.
                s_pss = [
                    pss.tile([128, TILE], F32, tag="pss", name="s_ps")
                    for _ in range(2)
                ]
                for pr in range(PAIRS):
                    for sb in (0, 1):
                        nc.tensor.matmul(
                            s_pss[sb],
                            w2[:, sb, pr, :, :],
                            ht[:, 2 * pr : 2 * pr + 2, :],
                            start=(pr == 0),
                            stop=(pr == PAIRS - 1),
                            perf_mode=DR,
                        )
                sn0s, sqs = [], []
                for sb in range(SB):
                    if sb < 2:
                        s_ps = s_pss[sb]
                    else:
                        s_ps = pss.tile(
                            [128, TILE], F32, tag="pss", name="s_ps"
                        )
                        for pr in range(PAIRS):
                            nc.tensor.matmul(
                                s_ps,
                                w2[:, sb, pr, :, :],
                                ht[:, 2 * pr : 2 * pr + 2, :],
                                start=(pr == 0),
                                stop=(pr == PAIRS - 1),
                                perf_mode=DR,
                            )
                    sn0 = sn0p.tile([128, TILE], BF16, tag="sn0", name="sn0")
                    if sb == 0:
                        nc.scalar.activation(
                            out=sn0, in_=s_ps, func=AF.Identity,
                            bias=b2t[:, sb : sb + 1],
                        )
                    else:
                        nc.vector.tensor_scalar_add(sn0, s_ps, b2t[:, sb : sb + 1])
                    sq = sqp.tile([128, TILE], F8, tag="sq", name="sq")
                    # Pool TT measures ~1097ns vs DVE's 403, but the 3
                    # squares fit its budget and it's otherwise idle (it
                    # cannot read PSUM, so sq/sn are all it can take).
                    # Last tile: DVE, to shorten the drain chain.
                    if t == nt - 1:
                        nc.vector.tensor_mul(sq, sn0, sn0)
                    else:
                        nc.gpsimd.tensor_mul(sq, sn0, sn0)
                    sn0s.append(sn0)
                    sqs.append(sq)
                return (t, sn0s, sqs)

            def stage_a(t):
                xt, ht = stage_a_mm1(t)
                return stage_a_mm2(t, ht)

            def stage_b1(st):
                # tile t: norm matmuls, rsqrt, sn = sn0*rin (produce the
                # normalized doc sub-vectors; consumed by b2 next iter)
                tp, sn0s, sqs = st
                sns = []
                for sb in range(SB):
                    n_ps = psn.tile([128, TILE], F32, tag="psn", name="n_ps")
                    nc.tensor.matmul(n_ps, mask, sqs[sb])
                    rin = rip.tile([128, TILE], BF16, tag="rin", name="rin")
                    _act_rsqrt(nc, rin, n_ps, epst[:, 0:1])
                    sn = snp.tile([128, TILE], BF16, tag="sn", name="sn")
                    # 2 of 3 on Pool: keeps the DVE queue short so the
                    # next tile's relu (gating MM2-DR) isn't delayed; sn
                    # itself has a full iteration of slack before b2.
                    if sb < 2 and tp != nt - 1:
                        nc.gpsimd.tensor_mul(sn, sn0s[sb], rin)
                    else:
                        nc.vector.tensor_mul(sn, sn0s[sb], rin)
                    if tp == 0:
                        # tile 0 cols 0:B are the normalized query subs
                        nc.vector.tensor_mul(
                            qmodT[:, sb, :], sn[:, 0:B], alphs[sb]
                        )
                    sns.append(sn)
                return (tp, sns)

            def stage_b2(st):
                # tile t: score matmuls + output copy + store
                tp, sns = st
                sc_ps = psc.tile([B, TILE], F32, tag="psc", name="sc_ps")
                for sb in range(SB):
                    nc.tensor.matmul(
                        sc_ps, qmodT[:, sb, :], sns[sb],
                        start=(sb == 0), stop=(sb == SB - 1),
                    )
                ot = outp.tile([B, TILE], F32, tag="ot", name="ot")
                if tp == nt - 1:
                    # last tile: chunk the copy+store so the final DMA
                    # overlaps the final copy instead of following it
                    for ch in range(2):
                        sl = slice(ch * (TILE // 2), (ch + 1) * (TILE // 2))
                        nc.vector.tensor_copy(ot[:, sl], sc_ps[:, sl])
                        nc.sync.dma_start(
                            out=scores[:, tp * TILE + ch * (TILE // 2) :
                                       tp * TILE + (ch + 1) * (TILE // 2)],
                            in_=ot[:, sl],
                        )
                else:
                    nc.vector.tensor_copy(ot, sc_ps)
                    nc.sync.dma_start(
                        out=scores[:, tp * TILE : (tp + 1) * TILE], in_=ot
                    )

            # Tile 0's MLP is emitted BEFORE the alpha phase so the PE
            # starts on doc matmuls (needing only w1 + xt0) instead of
            # head-of-line blocking on the query chain. The deferred
            # const DMAs slot between MM1 and MM2 so their Sync-queue
            # triggers trail the critical first wave.
            xt0 = xt_pre[0]
            _xt0_ref, ht0 = stage_a_mm1(0)
            nc.sync.dma_start(out=w2t[0:64, :], in_=w2pack[0:64, :])
            nc.sync.dma_start(out=w2t[64:128, :], in_=w2pack[64:128, :])
            nc.sync.dma_start(out=wrt, in_=wrest[:, :])
            nc.sync.dma_start(out=ct, in_=cpack_d[:, :])
            nc.sync.dma_start(out=mt, in_=mask8_d[:, :])
            a_out = {0: stage_a_mm2(0, ht0)}

            # ---- alpha phase: attention MLP on the query columns of
            # tile 0 (everything else about queries comes from the doc
            # pipeline itself). Produces alphs[sb] = alpha replicated to
            # the 128 partitions of s-block sb.
            aq_ps = psh.tile([AH, B], F32, tag="psh")
            for eb in range(EB):
                nc.tensor.matmul(
                    aq_ps, wa1[:, eb, :], xt0[:, eb, 0:B],
                    start=(eb == 0), stop=(eb == EB - 1),
                )
            aq = qpool.tile([AH, B], BF16)
            nc.scalar.activation(out=aq, in_=aq_ps, func=AF.Relu, bias=ba1t[:, 0:1])

            lq_ps = pss.tile([NSLOT, B], F32, tag="pss")
            nc.tensor.matmul(lq_ps, wa2, aq)
            eq = qpool.tile([NSLOT, B], BF16)
            nc.scalar.activation(out=eq, in_=lq_ps, func=AF.Exp, bias=ba2t[:, 0:1])

            sum_ps = psn.tile([128, B], F32, tag="psn")
            nc.tensor.matmul(sum_ps, ones6, eq)
            rsum = qpool.tile([128, B], F32)
            nc.vector.reciprocal(rsum, sum_ps)

            alphs = []
            for sb in range(SB):
                al_ps = psc.tile([128, B], F32, tag="psc")
                nc.tensor.matmul(al_ps, sel[:, sb * 128 : (sb + 1) * 128], eq)
                alph = qpool.tile([128, B], F32, tag="alph", name="alph")
                nc.vector.tensor_mul(alph, al_ps, rsum)
                alphs.append(alph)

            qmodT = consts.tile([128, SB, B], BF16)

            # ---- doc loop: per iteration emit a(t), b1(t-1), b2(t-2).
            # Every cross-engine hop (sq->norm, rsqrt->sn, sn->score)
            # gets a full tile-period of slack so the PE never waits.
            b_out = {}
            for t in range(1, nt + 2):
                if t < nt:
                    a_out[t] = stage_a(t)
                if 1 <= t <= nt:
                    b_out[t - 1] = stage_b1(a_out.pop(t - 1))
                if t >= 2:
                    stage_b2(b_out.pop(t - 2))

    nc.compile()
    return nc


def kernel(
    query_emb, doc_emb, W1, b1, W2, b2, Wa1, ba1, Wa2, ba2
):
    if "nc" not in _CACHE:
        _CACHE["nc"] = build()
    nc = _CACHE["nc"]

    bf = ml_dtypes.bfloat16
    f8 = ml_dtypes.float8_e4m3
    docs_t = np.zeros((E, N_CORES * D_CORE), dtype=bf)
    docs_t[:, :P_FULL] = doc_emb.reshape(P_FULL, E).T.astype(bf)
    q_t = np.ascontiguousarray(query_emb.reshape(B, E).T.astype(bf))

    w1pack = np.zeros((128, EB * H), dtype=bf)
    w1f = np.asarray(W1, dtype=np.float32)
    for eb in range(EB):
        w1pack[:, eb * H : (eb + 1) * H] = w1f[eb * 128 : (eb + 1) * 128].astype(bf)

    # SwInterleave image: w2pack[p, (sb*2+pr)*256 + k*2 + i] =
    # W2[(2*pr+i)*128 + p, sb*128 + (127-k)]
    w2f = np.asarray(W2, dtype=np.float32).reshape(PAIRS, 2, 128, SB, 128)
    # w2f[pair, i, p, sb, m] -> [p, sb, pair, m_rev, i]
    w2r = w2f[:, :, :, :, ::-1].transpose(2, 3, 0, 4, 1)
    w2pack = np.ascontiguousarray(
        w2r.reshape(128, SB * PAIRS * 2 * 128)
    ).astype(f8)

    wrest = np.zeros((128, EB * AH + NSLOT), dtype=bf)
    wa1f = np.asarray(Wa1, dtype=np.float32)
    wa2f = np.asarray(Wa2, dtype=np.float32)
    for eb in range(EB):
        wrest[:, eb * AH : (eb + 1) * AH] = wa1f[
            eb * 128 : (eb + 1) * 128
        ].astype(bf)
    wrest[:AH, EB * AH :] = wa2f.astype(bf)

    bpack = np.zeros((128, HB + SB + 2), dtype=np.float32)
    bpack[:, :HB] = np.asarray(b1, np.float32).reshape(HB, 128).T
    bpack[:, HB : HB + SB] = np.asarray(b2, np.float32).reshape(SB, 128).T
    bpack[:AH, HB + SB] = np.asarray(ba1, np.float32)
    bpack[:NSLOT, HB + SB + 1] = np.asarray(ba2, np.float32)

    common = {
        "w1pack": w1pack,
        "w2pack": w2pack,
        "wrest": wrest,
        "bpack": bpack,
    }
    in_maps = []
    for i in range(N_CORES):
        m = dict(common)
        full = np.concatenate(
            [q_t, docs_t[:, i * D_CORE : (i + 1) * D_CORE]], axis=1
        )
        # tile-major: docs[t*128+p, eb*TILE+c] = full[eb*128+p, t*TILE+c]
        m["docs"] = np.ascontiguousarray(
            full.reshape(EB, 128, NT, TILE)
            .transpose(2, 1, 0, 3)
            .reshape(NT * 128, EB * TILE)
        )
        in_maps.append(m)

    trace = _CACHE.get("trace", False)
    try:
        res = run_bass_kernel_spmd(
            nc, in_maps, core_ids=list(range(N_CORES)), trace=trace
        )
    except Exception:
        # rare transient NRT_EXEC_UNIT_UNRECOVERABLE on a freshly wedged
        # device; one retry has always succeeded
        res = run_bass_kernel_spmd(
            nc, in_maps, core_ids=list(range(N_CORES)), trace=False
        )
    _CACHE["last_result"] = res

    out = np.concatenate(
        [res.results[i]["scores"][:, B:] for i in range(N_CORES)], axis=1
    )
    return out[:, :P_FULL]


# revision 37
# speedup vs baseline: 1.0048x; 1.0048x over previous
"""DRT scorer kernel for Trainium2 (8 NeuronCores, Bass/Tile).

score[b, p] = sum_k alpha[b,k] * <qsub[b,k,:], dsub[p,k,:]>
with qsub/dsub per-slot-L2-normalized outputs of a shared 2-layer MLP
(E=384 -> H=512 -> K*SUB=384) and alpha a softmax over an attention MLP.

Strategy:
  - Fold alpha and query norms into the query side: qmod[b, s] =
    alpha[b, s//64] * qsub_norm[b, s].  Then score = Dnorm @ qmod.T.
  - Shard docs P across 8 cores (data parallel), pad 100000 -> 101888
    (12736/core; +64 query columns = 12800 = 25 tiles x 512).
  - MM1 (E->H) in bf16; MM2 (H->K*SUB) in fp8e4m3 with DoubleRow
    double-pumping (H=512 = 4 k-tiles = 2 DoubleRow passes, 2x rate).
    fp8 on one layer keeps L2 rel err ~1.3e-2 (< 2e-2 gate); fp8 on
    both layers measured 1.93e-2 - too close.
  - Per-slot doc norms via a block-diagonal ones matmul (fp8 operands)
    producing partition-replicated norm^2; 1/sqrt via ACT Rsqrt.
  - Elementwise split across ACT (2 relu, 1 sn0, 3 rsqrt), DVE (2
    relu, 2 sn0, 3 sn-mul), Pool (3 sq-square, out copy) so no engine
    exceeds the PE's ~4.5us/tile.
  - Software pipeline: tile t's norm+score phase is emitted one tile
    late so the PE never stalls on the norm chain.
"""

import sys

sys.path.insert(0, "/opt/trn_rl_repo")

import ml_dtypes
import numpy as np
import concourse.bacc as bacc
import concourse.mybir as mybir
from concourse.tile import TileContext
from concourse.bass_utils import run_bass_kernel_spmd

F32 = mybir.dt.float32
BF16 = mybir.dt.bfloat16
F8 = mybir.dt.float8e4
AF = mybir.ActivationFunctionType
ALU = mybir.AluOpType
DR = mybir.MatmulPerfMode.DoubleRow

E, H, KSUB = 384, 512, 384
NSLOT, SUB = 6, 64
AH = 64
B = 64
P_FULL = 100000
N_CORES = 8
TILE = 512
P_SHARD = 12800  # columns per core = B queries + D_CORE docs
NT = P_SHARD // TILE  # 25
D_CORE = P_SHARD - B  # 12736 doc slots per core (8*12736 = 101888 >= P_FULL)
EB, HB, SB = E // 128, H // 128, KSUB // 128  # 3, 4, 3
PAIRS = HB // 2  # 2 DoubleRow k-tile pairs for MM2
EPS = 1e-12

_CACHE = {}


def _act_rsqrt(nc, out, in_, bias_ap):
    """out = 1/sqrt(in + bias) on the ACT engine.

    bass refuses AF.Rsqrt on accuracy grounds (~0.4% worst case); the
    score tolerance here is much looser and this keeps the doc loop on a
    single activation-table set (the DVE reciprocal alternative costs
    ~3.2us per 512-col tile, and sqrt/ln/exp sit in different table sets
    whose reloads cost ~2.7us each).
    """
    sc = nc.scalar
    ins = [
        sc.lower_ap(in_),
        sc.lower_ap(bias_ap),
        mybir.ImmediateValue(dtype=F32, value=1.0),
        mybir.ImmediateValue(dtype=F32, value=0.0),
    ]
    return sc.add_instruction(
        mybir.InstActivation(
            name=nc.get_next_instruction_name(),
            func=AF.Rsqrt,
            ins=ins,
            outs=[sc.lower_ap(out)],
        )
    )


def _consts():
    # mask[p, j] = 1 iff p//64 == j//64  (block-diagonal 64x64 ones)
    idx = np.arange(128)
    mask = (idx[:, None] // SUB == idx[None, :] // SUB).astype(np.float32)
    # sel[k, sb*128 + j] = 1 iff k == 2*sb + j//64
    sel = np.zeros((NSLOT, KSUB), dtype=np.float32)
    for sb in range(SB):
        for j in range(128):
            sel[2 * sb + j // SUB, sb * 128 + j] = 1.0
    ones6 = np.ones((NSLOT, 128), dtype=np.float32)
    return mask, sel, ones6


def build(nt=NT):
    p_shard = nt * TILE
    nc = bacc.Bacc()

    # column layout per core: [0:B] = query embeddings, [B:] = doc shard.
    # Tile 0's MLP+norm pipeline thereby computes the normalized query
    # sub-vectors for free; only the alpha MLP runs separately.
    # Tile-major DRAM layout: docs[t*128+p, eb*TILE+c] = emb[eb*128+p,
    # global col], so each tile's DMA is 128 x 3KB contiguous runs
    # (1KB strided runs measured ~3x slower at startup).
    docs = nc.declare_dram_parameter(
        "docs", [nt * 128, EB * TILE], BF16, isOutput=False
    )
    # weights pre-packed on host into the exact SBUF image. w1 rides its
    # own DMA so the first doc-tile matmuls aren't queued behind the rest.
    RCOLS = EB * AH + NSLOT
    w1pack = nc.declare_dram_parameter("w1pack", [128, EB * H], BF16, isOutput=False)
    # w2 as fp8 DoubleRow weight image: col = ((sb*2+pair)*2+i)*128 + m,
    # value = W2[(2*pair+i)*128 + p, sb*128 + m]
    w2pack = nc.declare_dram_parameter(
        "w2pack", [128, SB * PAIRS * 2 * 128], F8, isOutput=False
    )
    wrest = nc.declare_dram_parameter("wrest", [128, RCOLS], BF16, isOutput=False)
    # biases packed: cols [0:HB] b1, [HB:HB+SB] b2, [HB+SB] ba1, [HB+SB+1] ba2
    bpack = nc.declare_dram_parameter("bpack", [128, HB + SB + 2], F32, isOutput=False)
    scores = nc.declare_dram_parameter("scores", [B, p_shard], F32, isOutput=True)

    mask_np, sel_np, ones6_np = _consts()
    bf = ml_dtypes.bfloat16
    cpack_np = np.zeros((128, KSUB + 128), dtype=np.float32)
    cpack_np[:NSLOT, 0:KSUB] = sel_np
    cpack_np[:NSLOT, KSUB:] = ones6_np
    cpack_d = nc.inline_tensor(cpack_np.astype(bf), name="cpack_d")
    mask8_d = nc.inline_tensor(
        mask_np.astype(ml_dtypes.float8_e4m3), name="mask8_d"
    )

    with TileContext(nc) as tc:
        with (
            tc.tile_pool(name="consts", bufs=1) as consts,
            tc.tile_pool(name="qpool", bufs=1) as qpool,
            tc.tile_pool(name="xtp", bufs=6) as xtp,
            tc.tile_pool(name="htp", bufs=4) as htp,
            tc.tile_pool(name="sn0p", bufs=9) as sn0p,
            tc.tile_pool(name="sqp", bufs=9) as sqp,
            tc.tile_pool(name="rip", bufs=6) as rip,
            tc.tile_pool(name="snp", bufs=12) as snp,
            tc.tile_pool(name="outp", bufs=4) as outp,
            tc.tile_pool(name="psh", bufs=3, space="PSUM") as psh,
            tc.tile_pool(name="pss", bufs=2, space="PSUM") as pss,
            tc.tile_pool(name="psn", bufs=2, space="PSUM") as psn,
            tc.tile_pool(name="psc", bufs=1, space="PSUM") as psc,
        ):
            # ---- constants / weights to SBUF ----
            # w1 rides the first DMA in one shot (contiguous 3KB runs);
            # the first doc tiles follow immediately on their own queues.
            # First wave = ONLY what gates the first matmuls (bpack, w1,
            # xt0), split into partition quarters: each split owns a DMA
            # queue and the 16 HW engines round-robin across ACTIVE
            # queues, so the critical bytes get most of the early
            # (~100GB/s) aggregate. Everything else is emitted after
            # tile 0's MM1 so its Sync-engine triggers (and thus queue
            # activation) trail the critical wave.
            bt = consts.tile([128, HB + SB + 2], F32)
            nc.sync.dma_start(out=bt, in_=bpack[:, :])
            b1t = bt[:, 0:HB]
            b2t = bt[:, HB : HB + SB]
            ba1t = bt[:AH, HB + SB : HB + SB + 1]
            ba2t = bt[:NSLOT, HB + SB + 1 : HB + SB + 2]

            w1t = consts.tile([128, EB * H], BF16)
            for q in range(4):
                nc.sync.dma_start(
                    out=w1t[q * 32 : (q + 1) * 32, :],
                    in_=w1pack[q * 32 : (q + 1) * 32, :],
                )
            w1 = w1t[:, :].rearrange("p (eb h) -> p eb h", eb=EB)

            docs_r = docs[:, :].rearrange(
                "(t p) (eb c) -> t p eb c", p=128, eb=EB
            )
            xt_pre = {}
            xt0_t = xtp.tile([128, EB, TILE], BF16, tag="xt")
            for q in range(4):
                nc.sync.dma_start(
                    out=xt0_t[q * 32 : (q + 1) * 32],
                    in_=docs_r[0, q * 32 : (q + 1) * 32],
                )
            xt_pre[0] = xt0_t

            epst = consts.tile([128, 1], F32)
            nc.vector.memset(epst, EPS)

            # Deferred-wave tiles (DMAs emitted later, between phases)
            w2t = consts.tile([128, SB * PAIRS * 2 * 128], F8)
            w2 = w2t[:, :].rearrange(
                "p (sb pr i m) -> p sb pr i m", sb=SB, pr=PAIRS, i=2
            )
            wrt = consts.tile([128, RCOLS], BF16)
            wa1 = wrt[:, 0 : EB * AH].rearrange("p (eb a) -> p eb a", eb=EB)
            wa2 = wrt[:AH, EB * AH :]
            ct = consts.tile([128, KSUB + 128], BF16)
            sel = ct[:NSLOT, 0:KSUB]
            ones6 = ct[:NSLOT, KSUB:]
            mt = consts.tile([128, 128], F8)
            mask = mt[:, :]

            def stage_a_mm1(t):
                # tile t: load + MM1 (bf16) + relu -> ht (fp8)
                if t in xt_pre:
                    xt = xt_pre.pop(t)
                else:
                    xt = xtp.tile([128, EB, TILE], BF16, tag="xt", name="xt")
                    nc.sync.dma_start(out=xt, in_=docs_r[t])
                ht = htp.tile([128, HB, TILE], F8, tag="ht", name="ht")
                for hb in range(HB):
                    h_ps = psh.tile([128, TILE], F32, tag="psh", name="h_ps")
                    for eb in range(EB):
                        nc.tensor.matmul(
                            h_ps,
                            w1[:, eb, hb * 128 : (hb + 1) * 128],
                            xt[:, eb, :],
                            start=(eb == 0),
                            stop=(eb == EB - 1),
                        )
                    if hb < 2:
                        nc.scalar.activation(
                            out=ht[:, hb, :], in_=h_ps, func=AF.Relu,
                            bias=b1t[:, hb : hb + 1],
                        )
                    else:
                        nc.vector.tensor_scalar(
                            out=ht[:, hb, :], in0=h_ps,
                            scalar1=b1t[:, hb : hb + 1],
                            scalar2=0.0, op0=ALU.add, op1=ALU.max,
                        )
                return xt, ht

            def stage_a_mm2(t, ht):
                # tile t: MM2 (fp8 DoubleRow) + sn0 + sq.
                # DR pairs for sb0/sb1 are interleaved across their two
                # PSUM banks so each 256-row DR LDWEIGHTS (~190ns, no
                # FWL) hides under a full matmul window.
                s_pss = [
                    pss.tile([128, TILE], F32, tag="pss", name="s_ps")
                    for _ in range(2)
                ]
                for pr in range(PAIRS):
                    for sb in (0, 1):
                        nc.tensor.matmul(
                            s_pss[sb],
                            w2[:, sb, pr, :, :],
                            ht[:, 2 * pr : 2 * pr + 2, :],
                            start=(pr == 0),
                            stop=(pr == PAIRS - 1),
                            perf_mode=DR,
                        )
                sn0s, sqs = [], []
                for sb in range(SB):
                    if sb < 2:
                        s_ps = s_pss[sb]
                    else:
                        s_ps = pss.tile(
                            [128, TILE], F32, tag="pss", name="s_ps"
                        )
                        for pr in range(PAIRS):
                            nc.tensor.matmul(
                                s_ps,
                                w2[:, sb, pr, :, :],
                                ht[:, 2 * pr : 2 * pr + 2, :],
                                start=(pr == 0),
                                stop=(pr == PAIRS - 1),
                                perf_mode=DR,
                            )
                    sn0 = sn0p.tile([128, TILE], BF16, tag="sn0", name="sn0")
                    if sb == 0:
                        nc.scalar.activation(
                            out=sn0, in_=s_ps, func=AF.Identity,
                            bias=b2t[:, sb : sb + 1],
                        )
                    else:
                        nc.vector.tensor_scalar_add(sn0, s_ps, b2t[:, sb : sb + 1])
                    sq = sqp.tile([128, TILE], F8, tag="sq", name="sq")
                    # Pool TT measures ~1097ns vs DVE's 403, but the 3
                    # squares fit its budget and it's otherwise idle (it
                    # cannot read PSUM, so sq/sn are all it can take).
                    # Last tile: DVE, to shorten the drain chain.
                    if t == nt - 1:
                        nc.vector.tensor_mul(sq, sn0, sn0)
                    else:
                        nc.gpsimd.tensor_mul(sq, sn0, sn0)
                    sn0s.append(sn0)
                    sqs.append(sq)
                return (t, sn0s, sqs)

            def stage_a(t):
                xt, ht = stage_a_mm1(t)
                return stage_a_mm2(t, ht)

            def stage_b1(st):
                # tile t: norm matmuls, rsqrt, sn = sn0*rin (produce the
                # normalized doc sub-vectors; consumed by b2 next iter)
                tp, sn0s, sqs = st
                sns = []
                for sb in range(SB):
                    n_ps = psn.tile([128, TILE], F32, tag="psn", name="n_ps")
                    nc.tensor.matmul(n_ps, mask, sqs[sb])
                    rin = rip.tile([128, TILE], BF16, tag="rin", name="rin")
                    _act_rsqrt(nc, rin, n_ps, epst[:, 0:1])
                    sn = snp.tile([128, TILE], BF16, tag="sn", name="sn")
                    # 2 of 3 on Pool: keeps the DVE queue short so the
                    # next tile's relu (gating MM2-DR) isn't delayed; sn
                    # itself has a full iteration of slack before b2.
                    if sb < 2 and tp != nt - 1:
                        nc.gpsimd.tensor_mul(sn, sn0s[sb], rin)
                    else:
                        nc.vector.tensor_mul(sn, sn0s[sb], rin)
                    if tp == 0:
                        # tile 0 cols 0:B are the normalized query subs
                        nc.vector.tensor_mul(
                            qmodT[:, sb, :], sn[:, 0:B], alphs[sb]
                        )
                    sns.append(sn)
                return (tp, sns)

            def stage_b2(st):
                # tile t: score matmuls + output copy + store
                tp, sns = st
                sc_ps = psc.tile([B, TILE], F32, tag="psc", name="sc_ps")
                for sb in range(SB):
                    nc.tensor.matmul(
                        sc_ps, qmodT[:, sb, :], sns[sb],
                        start=(sb == 0), stop=(sb == SB - 1),
                    )
                ot = outp.tile([B, TILE], F32, tag="ot", name="ot")
                if tp == nt - 1:
                    # last tile: chunk the copy+store so the final DMA
                    # overlaps the final copy instead of following it
                    for ch in range(2):
                        sl = slice(ch * (TILE // 2), (ch + 1) * (TILE // 2))
                        nc.vector.tensor_copy(ot[:, sl], sc_ps[:, sl])
                        nc.sync.dma_start(
                            out=scores[:, tp * TILE + ch * (TILE // 2) :
                                       tp * TILE + (ch + 1) * (TILE // 2)],
                            in_=ot[:, sl],
                        )
                else:
                    nc.vector.tensor_copy(ot, sc_ps)
                    nc.sync.dma_start(
                        out=scores[:, tp * TILE : (tp + 1) * TILE], in_=ot
                    )

            # Tile 0's MLP is emitted BEFORE the alpha phase so the PE
            # starts on doc matmuls (needing only w1 + xt0) instead of
            # head-of-line blocking on the query chain. The deferred
            # const DMAs slot between MM1 and MM2 so their Sync-queue
            # triggers trail the critical first wave.
            xt0 = xt_pre[0]
            _xt0_ref, ht0 = stage_a_mm1(0)
            nc.sync.dma_start(out=w2t[0:64, :], in_=w2pack[0:64, :])
            nc.sync.dma_start(out=w2t[64:128, :], in_=w2pack[64:128, :])
            nc.sync.dma_start(out=wrt, in_=wrest[:, :])
            nc.sync.dma_start(out=ct, in_=cpack_d[:, :])
            nc.sync.dma_start(out=mt, in_=mask8_d[:, :])
            a_out = {0: stage_a_mm2(0, ht0)}

            # ---- alpha phase: attention MLP on the query columns of
            # tile 0 (everything else about queries comes from the doc
            # pipeline itself). Produces alphs[sb] = alpha replicated to
            # the 128 partitions of s-block sb.
            aq_ps = psh.tile([AH, B], F32, tag="psh")
            for eb in range(EB):
                nc.tensor.matmul(
                    aq_ps, wa1[:, eb, :], xt0[:, eb, 0:B],
                    start=(eb == 0), stop=(eb == EB - 1),
                )
            aq = qpool.tile([AH, B], BF16)
            nc.scalar.activation(out=aq, in_=aq_ps, func=AF.Relu, bias=ba1t[:, 0:1])

            lq_ps = pss.tile([NSLOT, B], F32, tag="pss")
            nc.tensor.matmul(lq_ps, wa2, aq)
            eq = qpool.tile([NSLOT, B], BF16)
            nc.scalar.activation(out=eq, in_=lq_ps, func=AF.Exp, bias=ba2t[:, 0:1])

            sum_ps = psn.tile([128, B], F32, tag="psn")
            nc.tensor.matmul(sum_ps, ones6, eq)
            rsum = qpool.tile([128, B], F32)
            nc.vector.reciprocal(rsum, sum_ps)

            alphs = []
            for sb in range(SB):
                al_ps = psc.tile([128, B], F32, tag="psc")
                nc.tensor.matmul(al_ps, sel[:, sb * 128 : (sb + 1) * 128], eq)
                alph = qpool.tile([128, B], F32, tag="alph", name="alph")
                nc.vector.tensor_mul(alph, al_ps, rsum)
                alphs.append(alph)

            qmodT = consts.tile([128, SB, B], BF16)

            # ---- doc loop: per iteration emit a(t), b1(t-1), b2(t-2).
            # Every cross-engine hop (sq->norm, rsqrt->sn, sn->score)
            # gets a full tile-period of slack so the PE never waits.
            b_out = {}
            for t in range(1, nt + 2):
                if t < nt:
                    a_out[t] = stage_a(t)
                if 1 <= t <= nt:
                    b_out[t - 1] = stage_b1(a_out.pop(t - 1))
                if t >= 2:
                    stage_b2(b_out.pop(t - 2))

    nc.compile()
    return nc


def kernel(
    query_emb, doc_emb, W1, b1, W2, b2, Wa1, ba1, Wa2, ba2
):
    if "nc" not in _CACHE:
        _CACHE["nc"] = build()
    nc = _CACHE["nc"]

    bf = ml_dtypes.bfloat16
    f8 = ml_dtypes.float8_e4m3
    docs_t = np.zeros((E, N_CORES * D_CORE), dtype=bf)
    docs_t[:, :P_FULL] = doc_emb.reshape(P_FULL, E).T.astype(bf)
    q_t = np.ascontiguousarray(query_emb.reshape(B, E).T.astype(bf))

    w1pack = np.zeros((128, EB * H), dtype=bf)
    w1f = np.asarray(W1, dtype=np.float32)
    for eb in range(EB):
        w1pack[:, eb * H : (eb + 1) * H] = w1f[eb * 128 : (eb + 1) * 128].astype(bf)

    # w2pack[p, ((sb*2+pair)*2+i)*128 + m] = W2[(2*pair+i)*128 + p, sb*128 + m]
    w2f = np.asarray(W2, dtype=np.float32).reshape(PAIRS, 2, 128, SB, 128)
    # w2f[pair, i, p, sb, m] -> [p, sb, pair, i, m]
    w2pack = np.ascontiguousarray(
        w2f.transpose(2, 3, 0, 1, 4).reshape(128, SB * PAIRS * 2 * 128)
    ).astype(f8)

    wrest = np.zeros((128, EB * AH + NSLOT), dtype=bf)
    wa1f = np.asarray(Wa1, dtype=np.float32)
    wa2f = np.asarray(Wa2, dtype=np.float32)
    for eb in range(EB):
        wrest[:, eb * AH : (eb + 1) * AH] = wa1f[
            eb * 128 : (eb + 1) * 128
        ].astype(bf)
    wrest[:AH, EB * AH :] = wa2f.astype(bf)

    bpack = np.zeros((128, HB + SB + 2), dtype=np.float32)
    bpack[:, :HB] = np.asarray(b1, np.float32).reshape(HB, 128).T
    bpack[:, HB : HB + SB] = np.asarray(b2, np.float32).reshape(SB, 128).T
    bpack[:AH, HB + SB] = np.asarray(ba1, np.float32)
    bpack[:NSLOT, HB + SB + 1] = np.asarray(ba2, np.float32)

    common = {
        "w1pack": w1pack,
        "w2pack": w2pack,
        "wrest": wrest,
        "bpack": bpack,
    }
    in_maps = []
    for i in range(N_CORES):
        m = dict(common)
        full = np.concatenate(
            [q_t, docs_t[:, i * D_CORE : (i + 1) * D_CORE]], axis=1
        )
        # tile-major: docs[t*128+p, eb*TILE+c] = full[eb*128+p, t*TILE+c]
        m["docs"] = np.ascontiguousarray(
            full.reshape(EB, 128, NT, TILE)
            .transpose(2, 1, 0, 3)
            .reshape(NT * 128, EB * TILE)
        )
        in_maps.append(m)

    trace = _CACHE.get("trace", False)
    try:
        res = run_bass_kernel_spmd(
            nc, in_maps, core_ids=list(range(N_CORES)), trace=trace
        )
    except Exception:
        # rare transient NRT_EXEC_UNIT_UNRECOVERABLE on a freshly wedged
        # device; one retry has always succeeded
        res = run_bass_kernel_spmd(
            nc, in_maps, core_ids=list(range(N_CORES)), trace=False
        )
    _CACHE["last_result"] = res

    out = np.concatenate(
        [res.results[i]["scores"][:, B:] for i in range(N_CORES)], axis=1
    )
    return out[:, :P_FULL]
